# revision 8
# baseline (speedup 1.0000x reference)
"""ALayer kernel for 8 TRN2 NeuronCores — pure data parallel over batch.

Per-core shard: 4 images of [256, 56, 56].
  h  = relu(conv3x3(x_in, w1))      # 256 -> 16 ch
  A  = sigmoid(conv3x3(h, w2))      # 16 -> 1 ch
  out = x_out * box3x3(A)           # broadcast over 256 ch

v7 design — DMA-roofline schedule (HBM ~16.3MB/core ≈ 46us floor):
  conv1: v6's column-tiled rounds (4 concurrent 32-col strips, fp8,
         M=16, N=392; 18 rounds x 2 supergroups per image).  Relu evacs
         write the h plane = hcol[0:16] directly (2 ACT + 2 DVE per s).
  hcol:  K-folded im2col for conv2 — 4 chained SBUF->SBUF DMAs build
         hcol[16t+m, i] = h1f[m, i + 58*dy + dx] for taps t=0..7
         (dx-shifts then dy-shifts re-reading hcol itself).  Tap 8 needs
         no copy: it is read straight off the plane with a (+2,+2) AP.
  conv2: per supergroup ONE K=128 col-tiled round (8 taps folded) plus
         one K=16 round (tap 8) -> 4 PE rounds per image (was 18).
  a9:    v6's scatter construction (6 scatters + 2 row-shifts, SWDGE).
  box:   7 blocks of 8 rows: K=9 ones matmul -> psum [128,8,56];
         evacs to bf16 ab split ACT/DVE; muls are bf16 tensor_tensor
         (DVE 2x mode) in two [128,2,1792] chunks -> ot.
  Engine split: loads on SP ring, stores on ACT ring, hcol+a9 SWDGE on
         Q7, PE stream software-pipelined across images (conv1(i+1)
         rounds fill conv2/box dep-latency of image i) with a short
         fp8 warm prologue and tail warm filler to keep HAM at 8/8.
"""

import numpy as np
import ml_dtypes

import concourse.bass as bass
import concourse.tile as tile
import concourse.mybir as mybir
from concourse import bacc
from concourse.bass_utils import run_bass_kernel_spmd

BF16 = mybir.dt.bfloat16
FP8 = mybir.dt.float8e4
F32 = mybir.dt.float32

B, C, H, W = 32, 256, 56, 56
NCORES = 8
BL = B // NCORES          # images per core
KCH = 2                   # 256 = 2 chunks of 128
HP = H + 2                # padded plane side (58)
HW = H * W                # 3136
PL = HP * HP              # 3364

_cache = {}


def _build():
    nc = bacc.Bacc("TRN2", target_bir_lowering=False, debug=False)

    xin_d = nc.dram_tensor("xin", [BL, KCH, 128, PL], FP8, kind="ExternalInput").ap()
    xout_d = nc.dram_tensor("xout", [BL, 128, KCH, HW], BF16, kind="ExternalInput").ap()
    w1_d = nc.dram_tensor("w1t", [128, KCH, 9, 16], FP8, kind="ExternalInput").ap()
    w2_d = nc.dram_tensor("w2t", [128, 2], BF16, kind="ExternalInput").ap()
    out_d = nc.dram_tensor("out", [BL, 128, KCH, HW], BF16, kind="ExternalOutput").ap()

    with tile.TileContext(nc) as tc:
        with (
            tc.tile_pool(name="const", bufs=1) as constp,
            tc.tile_pool(name="xpad", bufs=4) as xpadp,
            tc.tile_pool(name="hcol", bufs=2) as hcolp,
            tc.tile_pool(name="at", bufs=2) as atp,
            tc.tile_pool(name="a9", bufs=2) as a9p,
            tc.tile_pool(name="ab", bufs=2) as abp,
            tc.tile_pool(name="xo", bufs=4) as xop,
            tc.tile_pool(name="ot", bufs=2) as otp,
            tc.tile_pool(name="ps_h", bufs=2, space="PSUM") as ps_h,
            tc.tile_pool(name="ps_a", bufs=2, space="PSUM") as ps_a,
            tc.tile_pool(name="ps_b", bufs=3, space="PSUM") as ps_b,
            tc.tile_pool(name="ps_w", bufs=1, space="PSUM") as ps_w,
        ):
            w1sb = constp.tile([128, KCH, 9, 16], FP8)
            w2sb = constp.tile([128, 2], BF16)
            ones9 = constp.tile([9, 128], BF16)
            wl = constp.tile([128, 128], FP8)
            wr = constp.tile([128, 512], FP8)

            # ---- all HBM loads issued up front on the SP ring ----
            nc.sync.dma_start(w1sb[:], w1_d[:])
            nc.sync.dma_start(w2sb[:], w2_d[:])
            xpads, xos = [], []
            for img in range(BL):
                xpads.append(
                    xpadp.tile([128, KCH, HP, HP], FP8, name="xpad")
                )
                xos.append(xop.tile([128, KCH, HW], BF16, name="xo"))
            MID = 30 * HP

            def load_xin(img, split):
                xpf = xpads[img].rearrange("p k r w -> p k (r w)")
                if split:
                    for k in range(KCH):
                        nc.sync.dma_start(xpf[:, k, 0:MID], xin_d[img, k, :, 0:MID])
                    for k in range(KCH):
                        nc.sync.dma_start(xpf[:, k, MID:PL], xin_d[img, k, :, MID:PL])
                else:
                    for k in range(KCH):
                        nc.sync.dma_start(xpf[:, k, :], xin_d[img, k, :, :])

            load_xin(0, True)
            load_xin(1, True)
            nc.sync.dma_start(xos[0][:], xout_d[0])
            load_xin(2, False)
            nc.sync.dma_start(xos[1][:], xout_d[1])
            load_xin(3, False)
            nc.sync.dma_start(xos[2][:], xout_d[2])
            nc.sync.dma_start(xos[3][:], xout_d[3])

            # ---- warm-up / filler matmuls (fp8 N=512, no data deps) ----
            nc.vector.memset(ones9[:], 1.0)
            nc.gpsimd.memset(wl[:], 0.0)
            nc.gpsimd.memset(wr[:], 0.0)

            def warm(n):
                for _ in range(n):
                    wp = ps_w.tile([128, 512], F32)
                    nc.tensor.matmul(
                        wp[:], wl[:], wr[:],
                        start=True, stop=True, skip_group_check=True,
                    )

            hcols, a9s, ats = {}, {}, {}

            def gen_conv1(img):
                """36 PE rounds; relu evacs into the h plane (hcol[0:16])."""
                xpad = xpads[img]
                hcol = hcolp.tile([128, HP, HP], BF16)
                hcols[img] = hcol
                if img < 2:
                    nc.vector.memset(hcol[0:16, 0, :], 0.0)
                    nc.vector.memset(hcol[0:16, 57, :], 0.0)
                    nc.vector.memset(hcol[0:16, :, 0], 0.0)
                    nc.vector.memset(hcol[0:16, :, 57], 0.0)
                for s in range(2):
                    ps = ps_h.tile([128, 7, 56], F32)
                    rnd = 0
                    for k in range(KCH):
                        for t in range(9):
                            dy, dx = t // 3, t % 3
                            for j in range(4):
                                rs = 28 * s + j + dy
                                nc.tensor.matmul(
                                    ps[32 * j : 32 * j + 16],
                                    w1sb[:, k, t, :],
                                    xpad[:, k, rs : rs + 25 : 4, dx : dx + 56],
                                    start=(rnd == 0),
                                    stop=(rnd == 17),
                                    tile_position=(0, 32 * j),
                                    skip_group_check=True,
                                )
                            rnd += 1
                            if rnd == 18:
                                for j in range(4):
                                    r0 = 1 + 28 * s + j
                                    dst = hcol[0:16, r0 : r0 + 25 : 4, 1:57]
                                    if j < 2:
                                        nc.scalar.activation(
                                            dst, ps[32 * j : 32 * j + 16],
                                            mybir.ActivationFunctionType.Relu,
                                        )
                                    else:
                                        nc.vector.tensor_scalar_max(
                                            dst, ps[32 * j : 32 * j + 16], 0.0
                                        )
                            yield

            def emit_hcol(img):
                """4 chained SWDGE copies build taps 1..7 from the plane."""
                hf = hcols[img].rearrange("p r w -> p (r w)")
                nc.gpsimd.dma_start(hf[16:32, 0 : PL - 1], hf[0:16, 1:PL])
                nc.gpsimd.dma_start(hf[32:48, 0 : PL - 2], hf[0:16, 2:PL])
                nc.gpsimd.dma_start(hf[48:96, 0 : PL - 58], hf[0:48, 58:PL])
                nc.gpsimd.dma_start(hf[96:128, 0 : PL - 116], hf[0:32, 116:PL])

            def gen_conv2(img):
                """4 PE rounds; sigmoid evac; a9 scatter build (SWDGE)."""
                hcol = hcols[img]
                at = atp.tile([128, 2, 7, HP], BF16)
                a9 = a9p.tile([9, HP, HP], BF16)
                ats[img], a9s[img] = at, a9
                a9f = a9.rearrange("p r w -> p (r w)")
                if img < 2:
                    nc.vector.memset(at[:, :, :, 0], 0.0)
                    nc.vector.memset(at[:, :, :, 57], 0.0)
                    nc.vector.memset(a9[:, 0, :], 0.0)
                    nc.vector.memset(a9[:, 57, :], 0.0)
                    nc.vector.memset(a9[:, :, 0:2], 0.0)
                    nc.vector.memset(a9[:, :, 56:58], 0.0)
                for s in range(2):
                    ps = ps_a.tile([128, 7, 56], F32)
                    for j in range(4):
                        b = 4 * s + j
                        nc.tensor.matmul(
                            ps[32 * j : 32 * j + 1],
                            w2sb[:, 0:1],
                            hcol[:, 7 * b : 7 * b + 7, 0:56],
                            start=True, stop=False,
                            tile_position=(0, 32 * j),
                            skip_group_check=True,
                        )
                    yield
                    for j in range(4):
                        b = 4 * s + j
                        nc.tensor.matmul(
                            ps[32 * j : 32 * j + 1],
                            w2sb[0:16, 1:2],
                            hcol[0:16, 7 * b + 2 : 7 * b + 9, 2:58],
                            start=False, stop=True,
                            tile_position=(0, 32 * j),
                            skip_group_check=True,
                        )
                    nc.scalar.activation(
                        at[:, s, :, 1:57], ps[:],
                        mybir.ActivationFunctionType.Sigmoid,
                    )
                    if s == 1:
                        for c in range(3):
                            for s2 in range(2):
                                st = (1 + 28 * s2) * HP + (1 - c)
                                nc.gpsimd.dma_start(
                                    a9f[3 + c : 4 + c, st : st + 1624],
                                    at[0:128:32, s2],
                                )
                        nc.gpsimd.dma_start(
                            a9f[0:3, HP : 57 * HP], a9f[3:6, 0 : 56 * HP]
                        )
                        nc.gpsimd.dma_start(
                            a9f[6:9, HP : 57 * HP], a9f[3:6, 2 * HP : PL]
                        )
                    yield

            def gen_box(img):
                """7 blocks of 8 rows; evacs split ACT/DVE; bf16 muls."""
                a9 = a9s[img]
                xo = xos[img]
                ab = abp.tile([128, 56, 56], BF16)
                abf = ab.rearrange("p r w -> p (r w)")
                ot = otp.tile([128, KCH, HW], BF16)

                def halfdone(h):
                    s0, s1 = (0, 1792) if h == 0 else (1792, HW)
                    nc.vector.tensor_mul(
                        ot[:, :, s0:s1],
                        xo[:, :, s0:s1],
                        abf[:, s0:s1].unsqueeze(1).broadcast_to(
                            [128, KCH, s1 - s0]
                        ),
                    )
                    nc.scalar.dma_start(
                        out_d[img, :, :, s0:s1], ot[:, :, s0:s1]
                    )

                for R in range(7):
                    psb = ps_b.tile([128, 8, 56], F32)
                    nc.tensor.matmul(
                        psb[:], ones9[:],
                        a9[:, 1 + 8 * R : 9 + 8 * R, 1:57],
                        start=True, stop=True,
                    )
                    dst = ab[:, 8 * R : 8 * R + 8, :]
                    if R % 2 == 0:
                        nc.scalar.activation(
                            dst, psb[:], mybir.ActivationFunctionType.Copy
                        )
                    else:
                        nc.vector.tensor_copy(dst, psb[:])
                    if R == 3:
                        halfdone(0)
                    elif R == 6:
                        halfdone(1)
                    yield

            def run(gen, n):
                for _ in range(n):
                    next(gen, None)

            c1 = [gen_conv1(i) for i in range(BL)]
            c2 = [gen_conv2(i) for i in range(BL)]
            bx = [gen_box(i) for i in range(BL)]

            warm(6)
            run(c1[0], 36)
            emit_hcol(0)
            run(c1[1], 22)
            run(c2[0], 4)
            run(c1[1], 14)
            emit_hcol(1)
            run(c1[2], 9)
            # box(0) interleaved 1:3 into conv1(2)
            for R in range(7):
                run(bx[0], 1)
                run(c1[2], 3)
            run(c2[1], 4)
            run(c1[2], 6)
            emit_hcol(2)
            run(c1[3], 9)
            for R in range(7):
                run(bx[1], 1)
                run(c1[3], 3)
            run(c2[2], 4)
            run(c1[3], 6)
            emit_hcol(3)
            warm(6)
            run(bx[2], 7)
            run(c2[3], 4)
            warm(4)
            run(bx[3], 7)

    nc.compile()
    return nc


def _prep_shards(x_in, x_out, w1, w2):
    bf16 = ml_dtypes.bfloat16
    fp8 = ml_dtypes.float8_e4m3
    # w1t[c, k, t, m] = w1[m, 128k + c, dy, dx],  t = 3*dy + dx
    w1t = np.ascontiguousarray(
        w1.reshape(16, KCH, 128, 9).transpose(2, 1, 3, 0)
    ).astype(fp8)
    # w2t col 0: K-folded taps 0..7 -> w2t[16t + m, 0] = w2[0, m, dy, dx]
    # w2t col 1: tap 8 at partitions 0..15
    w2t = np.zeros((128, 2), dtype=bf16)
    w2r = w2[0].reshape(16, 9)
    for t in range(8):
        w2t[16 * t : 16 * t + 16, 0] = w2r[:, t].astype(bf16)
    w2t[0:16, 1] = w2r[:, 8].astype(bf16)
    xi = np.zeros((NCORES, BL, KCH, 128, HP, HP), dtype=fp8)
    xi[..., 1 : 1 + H, 1 : 1 + W] = (
        x_in.reshape(NCORES, BL, KCH, 128, H, W).astype(fp8)
    )
    xi = xi.reshape(NCORES, BL, KCH, 128, PL)
    # xout[img, c_partition, k, hw]
    xo = np.ascontiguousarray(
        x_out.reshape(NCORES, BL, KCH, 128, HW).transpose(0, 1, 3, 2, 4)
    ).astype(bf16)
    return [
        {
            "xin": np.ascontiguousarray(xi[i]),
            "xout": xo[i],
            "w1t": w1t,
            "w2t": w2t,
        }
        for i in range(NCORES)
    ]


def _run(in_maps, trace=False):
    if "nc" not in _cache:
        _cache["nc"] = _build()
    return run_bass_kernel_spmd(
        _cache["nc"], in_maps, core_ids=list(range(NCORES)), trace=trace
    )


def kernel(x_in, x_out, w1, w2, _trace=False):
    in_maps = _prep_shards(
        np.asarray(x_in, dtype=np.float32),
        np.asarray(x_out, dtype=np.float32),
        np.asarray(w1, dtype=np.float32),
        np.asarray(w2, dtype=np.float32),
    )
    res = _run(in_maps, trace=_trace)
    # out[img, c_partition, k, hw] bf16 -> [B, C, H, W] fp32
    out = np.stack([res.results[i]["out"] for i in range(NCORES)])
    kernel.last_exec_time_ns = res.exec_time_ns
    out = out.astype(np.float32).transpose(0, 1, 3, 2, 4)
    return out.reshape(B, C, H, W)
